# revision 5
# baseline (speedup 1.0000x reference)
"""Llama2 attention layer (dense, fp32) on 8 Trainium2 NeuronCores.

Tensor-parallel across heads: each core owns 4 of the 32 heads (512 of the
4096 e-dims). wq/wk/wv are sharded column-wise, wo row-wise; the o-proj
partial sums are reduced on the host. All matmuls run in float32r (full PE
speed, ~1e-4 matmul relative error).

Device layouts (per core):
  x^T      (4096_d, 2048_s)  - contraction dims on partitions
  Q^T/K^T  (128_hd, 2048_s) per head, hd pre-permuted to [evens | odds]
  V        (s, 512_e) natural; re-read per head as (128_key, kt, 128_hd)
  scores^T (128_key, 512_s) tiles; softmax denominator via ones-matmul
"""
import sys, os, math
for _p in ("/opt/trn_rl_repo", "/root/.axon_site/_ro/trn_rl_repo"):
    if os.path.isdir(_p) and _p not in sys.path:
        sys.path.insert(0, _p)

import numpy as np
import concourse.bacc as bacc
import concourse.mybir as mybir
import concourse.tile as tile
from concourse.bass_utils import run_bass_kernel_spmd

F32R = mybir.dt.float32r
F32 = mybir.dt.float32
P = 128
N_CORES = 8
H = 4              # heads per core
HD = 128
D = 4096
S = 2048
PAST = 1024
TOT = PAST + S
NST = 4            # s-tiles of 512 over the 2048 queries
ST = 512
NKT = TOT // P     # 24 key tiles
SCALE = 1.0 / math.sqrt(HD)
HALF = 1024        # phase-1 x residency chunk


def _phase1(tc, nc, xq_src, wqTr, wkTr, wvTr, cosP, sinP, qT_out, kT_out,
            vor, j_sb):
    """Projections + rope. Writes qT_out/kT_out/v_out DRAM."""
    with (
        tc.tile_pool(name="xq", bufs=1) as xpool,
        tc.tile_pool(name="wstream", bufs=3) as wpool,
        tc.tile_pool(name="cospool", bufs=1) as cpool2,
        tc.tile_pool(name="tpool", bufs=9) as tpool,
        tc.tile_pool(name="rope", bufs=2) as rpool,
        tc.tile_pool(name="p1psum", bufs=8, space="PSUM") as p1ps,
    ):
        for half in range(2):
            s0 = half * HALF
            xq = xpool.tile([P, D // P, HALF], F32R, tag="xq")
            for dt in range(D // P):
                nc.sync.dma_start(xq[:, dt, :], xq_src[:, dt, s0:s0 + HALF])
            cos_sb = cpool2.tile([P, HALF], F32R, tag="cos")
            sin_sb = cpool2.tile([P, HALF], F32R, tag="sin")
            nc.sync.dma_start(cos_sb[:], cosP[:, s0:s0 + HALF])
            nc.sync.dma_start(sin_sb[:], sinP[:, s0:s0 + HALF])

            for wTr, outT in ((wqTr, qT_out), (wkTr, kT_out)):
                # 8 PSUM banks: (h, sc) for sc in {0,1}
                psb = [p1ps.tile([P, ST], F32, tag="projps", name=f"psb{i}")
                       for i in range(2 * H)]
                for dt in range(D // P):
                    w_sb = wpool.tile([P, H * HD], F32R, tag="wqk")
                    nc.sync.dma_start(w_sb[:], wTr[:, dt, :])
                    for h in range(H):
                        for sc in range(2):
                            nc.tensor.matmul(
                                psb[h * 2 + sc][:],
                                w_sb[:, h * HD:(h + 1) * HD],
                                xq[:, dt, sc * ST:(sc + 1) * ST],
                                start=(dt == 0), stop=(dt == D // P - 1))
                # rope: copy psum->sbuf first (frees banks), then J-swap
                t_sbs = []
                for i in range(2 * H):
                    t_sb = tpool.tile([P, ST], F32R, tag="t", name=f"t_sb{i}")
                    nc.vector.tensor_copy(out=t_sb[:], in_=psb[i][:])
                    t_sbs.append(t_sb)
                for h in range(H):
                    for sc in range(2):
                        ss = s0 + sc * ST
                        t_sb = t_sbs[h * 2 + sc]
                        jq = p1ps.tile([P, ST], F32, tag="projps")
                        nc.tensor.matmul(jq[:], j_sb[:], t_sb[:],
                                         start=True, stop=True)
                        t1 = rpool.tile([P, ST], F32R, tag="t1")
                        nc.vector.tensor_mul(
                            out=t1[:], in0=t_sb[:],
                            in1=cos_sb[:, sc * ST:(sc + 1) * ST])
                        t2 = rpool.tile([P, ST], F32R, tag="t2")
                        nc.vector.tensor_mul(
                            out=t2[:], in0=sin_sb[:, sc * ST:(sc + 1) * ST],
                            in1=jq[:])
                        rot = rpool.tile([P, ST], F32R, tag="rot", name="rot")
                        nc.vector.tensor_add(out=rot[:], in0=t1[:], in1=t2[:])
                        nc.sync.dma_start(outT[h, :, ss:ss + ST], rot[:])

            # V projection: natural layout, 8 banks (8 s-subtiles)
            psv = [p1ps.tile([P, H * HD], F32, tag="projps", name=f"psv{i}")
                   for i in range(HALF // P)]
            for dt in range(D // P):
                wv_sb = wpool.tile([P, H * HD], F32R, tag="wv")
                nc.sync.dma_start(wv_sb[:], wvTr[:, dt, :])
                for ss in range(HALF // P):
                    nc.tensor.matmul(
                        psv[ss][:],
                        xq[:, dt, ss * P:(ss + 1) * P],
                        wv_sb[:],
                        start=(dt == 0), stop=(dt == D // P - 1))
            for ss in range(HALF // P):
                vb = rpool.tile([P, H * HD], F32R, tag="vb")
                nc.vector.tensor_copy(out=vb[:], in_=psv[ss][:])
                nc.sync.dma_start(vor[:, half * 8 + ss, :], vb[:])


def _phase23(tc, nc, sched, kcT, kT_out, qT_out, vcr, vor, maskTr, woTr,
             opr, ones_sb):
    with (
        tc.tile_pool(name="kv", bufs=1) as kvpool,
        tc.tile_pool(name="attn", bufs=1) as apool,
    ):
        kt_res, v_res = [], []
        for h in range(H):
            kt_h = kvpool.tile([P, NKT * P], F32R, tag=f"kt{h}")
            nc.sync.dma_start(kt_h[:, 0:PAST], kcT[h])
            nc.sync.dma_start(kt_h[:, PAST:TOT], kT_out[h])
            kt_res.append(kt_h)
            v_h = kvpool.tile([P, NKT, HD], F32R, tag=f"v{h}")
            nc.sync.dma_start(v_h[:, 0:PAST // P, :],
                              vcr[:, :, h * HD:(h + 1) * HD])
            nc.sync.dma_start(v_h[:, PAST // P:, :],
                              vor[:, :, h * HD:(h + 1) * HD])
            v_res.append(v_h)

        attn = [[apool.tile([P, ST], F32R, tag=f"at{h}_{st}", name=f"attn{h}_{st}")
                 for st in range(NST)] for h in range(H)]

        with (
            tc.tile_pool(name="qstream", bufs=4) as qpool,
            tc.tile_pool(name="pt", bufs=4) as ppool,
            tc.tile_pool(name="msk", bufs=4) as mpool,
            tc.tile_pool(name="zr", bufs=3) as zpool,
            tc.tile_pool(name="scps", bufs=2, space="PSUM") as scps,
            tc.tile_pool(name="zps", bufs=2, space="PSUM") as zps,
            tc.tile_pool(name="ops", bufs=2, space="PSUM") as ops,
        ):
            for st in range(NST):
                kts, partial = sched[st]
                msk_sb = {}
                for kt in partial:
                    m = mpool.tile([P, ST], F32, tag="mask")
                    nc.sync.dma_start(
                        m[:], maskTr[:, kt, st * ST:(st + 1) * ST])
                    msk_sb[kt] = m
                for h in range(H):
                    q_sb = qpool.tile([P, ST], F32R, tag="q")
                    nc.sync.dma_start(q_sb[:],
                                      qT_out[h, :, st * ST:(st + 1) * ST])
                    z_ps = zps.tile([P, ST], F32, tag="z")
                    o_ps = ops.tile([P, ST], F32, tag="o")
                    pairs = [kts[i:i + 2] for i in range(0, len(kts), 2)]
                    for pi, pair in enumerate(pairs):
                        sc_ps = scps.tile([P, 2 * ST], F32, tag="sc")
                        for g, kt in enumerate(pair):
                            nc.tensor.matmul(
                                sc_ps[:, g * ST:(g + 1) * ST],
                                kt_res[h][:, kt * P:(kt + 1) * P],
                                q_sb[:], start=True, stop=True)
                        for g, kt in enumerate(pair):
                            if kt in partial:
                                nc.vector.tensor_add(
                                    out=sc_ps[:, g * ST:(g + 1) * ST],
                                    in0=sc_ps[:, g * ST:(g + 1) * ST],
                                    in1=msk_sb[kt][:])
                        p_sb = ppool.tile([P, 2 * ST], F32R, tag="p")
                        nc.scalar.activation(
                            p_sb[:, 0:len(pair) * ST],
                            sc_ps[:, 0:len(pair) * ST],
                            mybir.ActivationFunctionType.Exp, scale=SCALE)
                        first = (pi == 0)
                        last = (pi == len(pairs) - 1)
                        for g, kt in enumerate(pair):
                            lastg = last and (g == len(pair) - 1)
                            nc.tensor.matmul(
                                z_ps[:], ones_sb[:],
                                p_sb[:, g * ST:(g + 1) * ST],
                                start=(first and g == 0), stop=lastg)
                            nc.tensor.matmul(
                                o_ps[:], v_res[h][:, kt, :],
                                p_sb[:, g * ST:(g + 1) * ST],
                                start=(first and g == 0), stop=lastg)
                    zr = zpool.tile([P, ST], F32, tag="zr")
                    nc.vector.reciprocal(zr[:], z_ps[:])
                    nc.vector.tensor_mul(out=attn[h][st][:], in0=o_ps[:],
                                         in1=zr[:])

        # ---------------- Phase 3: o-projection ----------------
        with (
            tc.tile_pool(name="wo", bufs=3) as wopool,
            tc.tile_pool(name="obuf", bufs=4) as obpool,
            tc.tile_pool(name="oproj", bufs=4, space="PSUM") as oproj,
        ):
            for dt in range(D // P):
                wo_sb = wopool.tile([P, H, P], F32R, tag="wo")
                nc.sync.dma_start(wo_sb[:], woTr[:, :, dt * P:(dt + 1) * P])
                for st in range(NST):
                    po = oproj.tile([P, ST], F32, tag="po")
                    for h in range(H):
                        nc.tensor.matmul(po[:], wo_sb[:, h, :],
                                         attn[h][st][:],
                                         start=(h == 0), stop=(h == H - 1))
                    ob = obpool.tile([P, ST], F32, tag="ob")
                    nc.scalar.copy(ob[:], po[:])
                    nc.sync.dma_start(
                        opr[:, dt, st * ST:(st + 1) * ST], ob[:])


def build_nc(sched, repeat=1):
    """sched: list over st of (kts, partial_set); kts = key-tile indices to
    process, partial_set = subset needing a mask add."""
    nc = bacc.Bacc(None, target_bir_lowering=False)

    xT = nc.dram_tensor("xT", [D, S], F32R, kind="ExternalInput")
    wqT = nc.dram_tensor("wqT", [D, H * HD], F32R, kind="ExternalInput")
    wkT = nc.dram_tensor("wkT", [D, H * HD], F32R, kind="ExternalInput")
    wvT = nc.dram_tensor("wvT", [D, H * HD], F32R, kind="ExternalInput")
    woT = nc.dram_tensor("woT", [H * HD, D], F32R, kind="ExternalInput")
    cosP = nc.dram_tensor("cosP", [P, S], F32R, kind="ExternalInput")
    sinP = nc.dram_tensor("sinP", [P, S], F32R, kind="ExternalInput")
    kcT = nc.dram_tensor("kcT", [H, P, PAST], F32R, kind="ExternalInput")
    vc = nc.dram_tensor("vc", [PAST, H * HD], F32R, kind="ExternalInput")
    maskT = nc.dram_tensor("maskT", [TOT, S], F32, kind="ExternalInput")
    jmat = nc.dram_tensor("jmat", [P, P], F32R, kind="ExternalInput")
    ones = nc.dram_tensor("ones", [P, P], F32R, kind="ExternalInput")

    kT_out = nc.dram_tensor("kT_out", [H, P, S], F32R, kind="ExternalOutput")
    qT_out = nc.dram_tensor("qT_out", [H, P, S], F32R, kind="ExternalOutput")
    v_out = nc.dram_tensor("v_out", [S, H * HD], F32R, kind="ExternalOutput")
    o_part = nc.dram_tensor("o_part", [D, S], F32, kind="ExternalOutput")

    xTr = xT.rearrange("(dt p) s -> p dt s", p=P)        # (128, 32, 2048)
    wqTr = wqT.rearrange("(dt p) e -> p dt e", p=P)      # (128, 32, 512)
    wkTr = wkT.rearrange("(dt p) e -> p dt e", p=P)
    wvTr = wvT.rearrange("(dt p) e -> p dt e", p=P)
    woTr = woT.rearrange("(h p) d -> p h d", p=P)        # (128, 4, 4096)
    vcr = vc.rearrange("(kt p) e -> p kt e", p=P)        # (128, 8, 512)
    vor = v_out.rearrange("(st p) e -> p st e", p=P)     # (128, 16, 512)
    opr = o_part.rearrange("(dt p) s -> p dt s", p=P)    # (128, 32, 2048)
    maskTr = maskT.rearrange("(kt p) s -> p kt s", p=P)  # (128, 24, 2048)

    with tile.TileContext(nc) as tc:
        def body(_i=None):
            with tc.tile_pool(name="consts", bufs=1) as cpool:
                j_sb = cpool.tile([P, P], F32R)
                ones_sb = cpool.tile([P, P], F32R)
                nc.sync.dma_start(j_sb[:], jmat[:])
                nc.sync.dma_start(ones_sb[:], ones[:])
                _phase1(tc, nc, xTr, wqTr, wkTr, wvTr, cosP, sinP,
                        qT_out, kT_out, vor, j_sb)
                _phase23(tc, nc, sched, kcT, kT_out, qT_out, vcr, vor,
                         maskTr, woTr, opr, ones_sb)

        if repeat == 1:
            body()
        else:
            with tc.For_i(0, repeat, 1) as _i:
                body(_i)
    nc.finalize()
    return nc


def _schedule_from_mask(mask2d):
    """mask2d: (S, TOT) additive mask. Classify each (st, kt) tile."""
    sched = []
    for st in range(NST):
        m = mask2d[st * ST:(st + 1) * ST, :]  # (512, TOT)
        kts, partial = [], set()
        for kt in range(NKT):
            tile_m = m[:, kt * P:(kt + 1) * P]
            if np.all(tile_m <= -1e8):
                continue
            kts.append(kt)
            if np.any(tile_m != 0.0):
                partial.add(kt)
        sched.append((kts, partial))
    return sched


_NC_CACHE = {}


def _get_nc(sched_key, sched):
    if sched_key not in _NC_CACHE:
        _NC_CACHE[sched_key] = build_nc(sched)
    return _NC_CACHE[sched_key]


def _perm():
    # within-head hd permutation: evens then odds
    return np.concatenate([np.arange(0, HD, 2), np.arange(1, HD, 2)])


def prepare_inputs(x, cos, sin, attention_mask, k_cache, v_cache, wq, wk, wv, wo):
    """Build the 8 per-core in_maps (all float32 numpy, device sees f32r)."""
    perm = _perm()
    xT = np.ascontiguousarray(x[0].T.astype(np.float32))          # (4096, 2048)
    cosT = np.ascontiguousarray(cos.T.astype(np.float32))         # (64, 2048)
    sinT = np.ascontiguousarray(sin.T.astype(np.float32))
    cosPa = np.concatenate([cosT, cosT], axis=0)                   # (128, 2048)
    sinPa = np.concatenate([sinT, sinT], axis=0)
    mask2d = attention_mask[0, 0].astype(np.float32)               # (2048, 3072)
    maskTa = np.ascontiguousarray(mask2d.T / SCALE)                # (3072, 2048)
    J = np.zeros((P, P), np.float32)
    for i in range(64):
        J[i, i + 64] = -1.0
        J[i + 64, i] = 1.0
    JT = np.ascontiguousarray(J.T)
    ONES = np.ones((P, P), np.float32)

    in_maps = []
    for c in range(N_CORES):
        rows = slice(c * H * HD, (c + 1) * H * HD)
        # per-head permuted rows for wq/wk
        idx = np.concatenate([c * H * HD + h * HD + perm for h in range(H)])
        wqTa = np.ascontiguousarray(wq[idx].T.astype(np.float32))  # (4096, 512)
        wkTa = np.ascontiguousarray(wk[idx].T.astype(np.float32))
        wvTa = np.ascontiguousarray(wv[rows].T.astype(np.float32))
        woTa = np.ascontiguousarray(wo[:, rows].T.astype(np.float32))
        kc = k_cache[0][:, c * H:(c + 1) * H, :]                  # (1024, 4, 128)
        kcTa = np.ascontiguousarray(
            kc[:, :, perm].transpose(1, 2, 0).astype(np.float32))  # (4, 128, 1024)
        vcs = np.ascontiguousarray(
            v_cache[0][:, c * H:(c + 1) * H, :].reshape(PAST, H * HD)
            .astype(np.float32))
        in_maps.append({
            "xT": xT, "wqT": wqTa, "wkT": wkTa, "wvT": wvTa, "woT": woTa,
            "cosP": cosPa, "sinP": sinPa, "kcT": kcTa, "vc": vcs,
            "maskT": maskTa, "jmat": JT, "ones": ONES,
        })
    return in_maps, mask2d


def assemble_outputs(results, k_cache, v_cache, x_dtype):
    perm = _perm()
    inv = np.argsort(perm)
    out = np.zeros((S, D), np.float32)
    k_new = np.zeros((1, TOT, 32, HD), np.float32)
    v_new = np.zeros((1, TOT, 32, HD), np.float32)
    k_new[0, :PAST] = np.asarray(k_cache[0])
    v_new[0, :PAST] = np.asarray(v_cache[0])
    for c, res in enumerate(results):
        out += res["o_part"].T
        kT = res["kT_out"]            # (4, 128, 2048) permuted hd
        for h in range(H):
            k_new[0, PAST:, c * H + h, :] = kT[h][inv].T
        v_new[0, PAST:, c * H:(c + 1) * H, :] = \
            res["v_out"].reshape(S, H, HD)
    return (out.reshape(1, S, D).astype(x_dtype),
            k_new.astype(x_dtype), v_new.astype(x_dtype))


def kernel(x, cos, sin, attention_mask, k_cache, v_cache, wq, wk, wv, wo,
           _return_raw=False):
    x = np.asarray(x); cos = np.asarray(cos); sin = np.asarray(sin)
    attention_mask = np.asarray(attention_mask)
    k_cache = np.asarray(k_cache); v_cache = np.asarray(v_cache)
    wq = np.asarray(wq); wk = np.asarray(wk); wv = np.asarray(wv)
    wo = np.asarray(wo)

    in_maps, mask2d = prepare_inputs(x, cos, sin, attention_mask, k_cache,
                                     v_cache, wq, wk, wv, wo)
    sched = _schedule_from_mask(mask2d)
    key = tuple((tuple(k), tuple(sorted(p))) for k, p in sched)
    nc = _get_nc(key, sched)
    results = run_bass_kernel_spmd(nc, in_maps, core_ids=list(range(N_CORES)))\
        .results
    outs = assemble_outputs(results, k_cache, v_cache, np.float32)
    if _return_raw:
        return outs, results
    return outs
